# revision 26
# baseline (speedup 1.0000x reference)
"""Trainium2 Bass kernel for a single attention head.

Problem: X[4,4096,1024], Wq/Wk/Wv[1024,128] ->
  softmax((X@Wq)(X@Wk)^T / sqrt(1024)) @ (X@Wv)   -> [4,4096,128]

Sharding: 8 cores = 4 batches x 2 query-halves. Each core receives the full
X of its batch (rolled so its query half is rows [0:2048)), computes K/V for
all 4096 keys and flash-style attention for its 2048 queries.

On-core algorithm (all matmuls bf16 inputs, fp32 PSUM accumulation):
  1. X -> bf16 (cast DMA) -> X^T via XBAR transpose-DMA.
  2. K^T[h,n], V^T[h,n], Q^T[h,q] projections; V^T -> V[k,h] via transpose-DMA.
  3. Transposed flash attention per 1024-query chunk:
       S^T[k,q] = K_tile @ Q^T   (PSUM)
       P^T = exp(S^T/32)         (ACT, bf16 out)
       O^T[h,q] += V_tile^T @ P^T  ;  l[1,q] += ones^T @ P^T
     Epilogue: PE-transpose O^T and l, scale by 1/l, DMA out.
"""

import numpy as np

B, N, D, H = 4, 4096, 1024, 128
NCORES = 8
QSPLIT = 2  # cores per batch (query halves)
NQ = N // QSPLIT
SCALE = 1.0 / float(np.sqrt(np.float32(D)))
P = 128  # partitions
FB = 512  # matmul free-dim block (one fp32 PSUM bank)


def emit_attention(tc, X, Wq, Wk, Wv, O, n=N, d=D, nq=NQ, qc=1024):
    """Emit the single-core attention program into TileContext tc.

    X: [n, d] f32 DRAM (queries are rows [0:nq)); W*: [d, H] f32; O: [nq, H] f32.
    """
    import concourse.mybir as mybir
    from concourse.masks import make_identity

    nc = tc.nc
    dt = mybir.dt
    f32, bf16 = dt.float32, dt.bfloat16
    AF = mybir.ActivationFunctionType

    DT = d // P   # d tiles (contraction tiles for projections)
    NT = n // P   # key tiles
    qc = min(qc, nq)
    QB = qc // P  # 128-query blocks per chunk
    assert nq % qc == 0 and d % P == 0 and n % P == 0 and qc % P == 0

    from contextlib import ExitStack

    with ExitStack() as ctx:
        cpool = ctx.enter_context(tc.tile_pool(name="const", bufs=1))
        big = ctx.enter_context(tc.tile_pool(name="big", bufs=1))
        ptp = ctx.enter_context(tc.tile_pool(name="pt", bufs=3))
        epp = ctx.enter_context(tc.tile_pool(name="ep", bufs=2))
        accsb = ctx.enter_context(tc.tile_pool(name="accsb", bufs=2))

        ident = cpool.tile([P, P], f32)
        make_identity(nc, ident[:])
        ones_f = cpool.tile([P, 1], f32)
        nc.gpsimd.memset(ones_f[:], 1.0)

        w_sb = {}
        for name, w in (("wq", Wq), ("wk", Wk), ("wv", Wv)):
            t = cpool.tile([P, DT * H], bf16, tag=name)
            nc.gpsimd.dma_start(
                t[:].rearrange("p (t h) -> p t h", t=DT),
                w.rearrange("(t p) h -> p t h", p=P),
            )
            w_sb[name] = t

        xt = big.tile([P, DT * n], bf16)    # X^T: [d%128, dt*n + ncol]
        kT = big.tile([P, n], bf16)         # K^T[h, n]
        qT = big.tile([P, nq], bf16)        # Q^T[h, q]
        vT = big.tile([P, n], bf16)         # V^T[h, n] (staging)
        v_sb = big.tile([P, NT * H], bf16)  # V[k%128, kt*H + h]

        # ---- Phases 1+2: load X (f32->bf16 cast DMA), transpose to X^T via
        # xbar DMA-transposes, then projections.
        LB = min(4, NT)  # n-row blocks per load DMA
        with (
            tc.tile_pool(name="xbfp", bufs=3) as xbf_pool,
            tc.tile_pool(name="p12", bufs=3, space="PSUM") as p12,
        ):
            for nt0 in range(0, NT, LB):
                xbf = xbf_pool.tile([P, LB * d], bf16, tag="xbf")
                nc.gpsimd.dma_start(
                    xbf[:].rearrange("p (a dd) -> p a dd", a=LB),
                    X[nt0 * P:(nt0 + LB) * P, :].rearrange(
                        "(a p) dd -> p a dd", p=P
                    ),
                )
                for a in range(LB):
                    nt = nt0 + a
                    for t in range(DT):
                        nc.sync.dma_start(
                            xt[:, t * n + nt * P: t * n + (nt + 1) * P],
                            xbf[:, a * d + t * P: a * d + (t + 1) * P],
                            transpose=True,
                        )

            def project(wname, dst, ncols):
                for c0 in range(0, ncols, FB):
                    w = min(FB, ncols - c0)
                    ps = p12.tile([P, FB], f32, tag="pps")
                    for t in range(DT):
                        nc.tensor.matmul(
                            ps[:, :w],
                            w_sb[wname][:, t * H:(t + 1) * H],
                            xt[:, t * n + c0: t * n + c0 + w],
                            start=(t == 0),
                            stop=(t == DT - 1),
                        )
                    nc.vector.tensor_copy(dst[:, c0:c0 + w], ps[:, :w])

            project("wk", kT, n)
            project("wv", vT, n)
            project("wq", qT, nq)

            # V^T -> V via xbar DMA-transposes
            for kt in range(NT):
                nc.sync.dma_start(
                    v_sb[:, kt * H:(kt + 1) * H],
                    vT[:, kt * P:(kt + 1) * P],
                    transpose=True,
                )

        # ---- Phase 3: attention ----
        with ExitStack() as actx:
            stp = actx.enter_context(tc.tile_pool(name="stps", bufs=2, space="PSUM"))
            accp = actx.enter_context(tc.tile_pool(name="accps", bufs=1, space="PSUM"))

            for q0 in range(0, nq, qc):
                out_ps = accp.tile([P, qc], f32, tag="out")
                l_ps = accp.tile([1, qc], f32, tag="l")
                acc = None
                for kt in range(NT):
                    st = stp.tile([P, qc], f32, tag="st")
                    for j in range(0, qc, FB):
                        w = min(FB, qc - j)
                        nc.tensor.matmul(
                            st[:, j:j + w],
                            kT[:, kt * P:(kt + 1) * P],
                            qT[:, q0 + j: q0 + j + w],
                            start=True, stop=True,
                        )
                    pT = ptp.tile([P, qc], bf16, tag="pt")
                    nc.scalar.activation(pT[:], st[:], AF.Exp, scale=SCALE)
                    for j in range(0, qc, FB):
                        w = min(FB, qc - j)
                        nc.tensor.matmul(
                            out_ps[:, j:j + w],
                            v_sb[:, kt * H:(kt + 1) * H],
                            pT[:, j:j + w],
                            start=(kt == 0), stop=(kt == NT - 1),
                        )
                    # softmax denominator: accumulate P^T on DVE (f32),
                    # reduced over partitions by one small matmul at the end
                    nacc = accsb.tile([P, qc], f32, tag="acc")
                    if kt == 0:
                        nc.vector.tensor_copy(nacc[:], pT[:])
                    else:
                        nc.vector.tensor_add(nacc[:], acc[:], pT[:])
                    acc = nacc
                for j in range(0, qc, FB):
                    w = min(FB, qc - j)
                    nc.tensor.matmul(
                        l_ps[:, j:j + w], ones_f[:], acc[:, j:j + w],
                        start=True, stop=True,
                    )

                # epilogue: 1/l, transpose O^T -> O, scale, store
                l_sb = epp.tile([1, qc], f32, tag="lsb")
                nc.vector.tensor_copy(l_sb[:], l_ps[:])
                r_sb = epp.tile([P, QB], f32, tag="rsb")
                for blk in range(QB):
                    lt = stp.tile([P, 1], f32, tag="st")
                    nc.tensor.transpose(
                        lt[:], l_sb[:, blk * P:(blk + 1) * P], ident[:1, :1]
                    )
                    nc.vector.reciprocal(r_sb[:, blk:blk + 1], lt[:])
                ob = epp.tile([P, qc], f32, tag="ob")
                nc.vector.tensor_copy(ob[:], out_ps[:])
                o_sb = epp.tile([P, QB * H], f32, tag="osb")
                for blk in range(QB):
                    ot = stp.tile([P, P], f32, tag="st")
                    nc.tensor.transpose(ot[:], ob[:, blk * P:(blk + 1) * P], ident[:])
                    nc.scalar.mul(
                        o_sb[:, blk * H:(blk + 1) * H], ot[:], r_sb[:, blk:blk + 1]
                    )
                nc.sync.dma_start(
                    O[q0:q0 + qc, :].rearrange("(qb p) h -> p qb h", p=P),
                    o_sb[:].rearrange("p (qb h) -> p qb h", qb=QB),
                )


def build_bass(n=N, d=D, nq=NQ, qc=1024):
    import concourse.mybir as mybir
    from concourse import bacc
    from concourse.tile import TileContext

    dt = mybir.dt
    nc = bacc.Bacc("TRN2", target_bir_lowering=False, debug=False)
    X = nc.dram_tensor("X", [n, d], dt.float32, kind="ExternalInput").ap()
    Wq = nc.dram_tensor("Wq", [d, H], dt.float32, kind="ExternalInput").ap()
    Wk = nc.dram_tensor("Wk", [d, H], dt.float32, kind="ExternalInput").ap()
    Wv = nc.dram_tensor("Wv", [d, H], dt.float32, kind="ExternalInput").ap()
    O = nc.dram_tensor("O", [nq, H], dt.float32, kind="ExternalOutput").ap()

    with TileContext(nc) as tc:
        emit_attention(tc, X, Wq, Wk, Wv, O, n=n, d=d, nq=nq, qc=qc)
    nc.compile()  # bacc passes: split multi-waits into EVSEM chains, etc.
    return nc


_CACHED = {}


def _get_nc():
    if "nc" not in _CACHED:
        _CACHED["nc"] = build_bass()
    return _CACHED["nc"]


def kernel(X, Wq, Wk, Wv, trace=False):
    """Full-input entry point: X [4,4096,1024] f32 -> [4,4096,128] f32."""
    from concourse.bass_utils import run_bass_kernel_spmd

    X = np.ascontiguousarray(X, dtype=np.float32)
    Wq = np.ascontiguousarray(Wq, dtype=np.float32)
    Wk = np.ascontiguousarray(Wk, dtype=np.float32)
    Wv = np.ascontiguousarray(Wv, dtype=np.float32)

    nc = _get_nc()
    in_maps = []
    for core in range(NCORES):
        b, half = core // QSPLIT, core % QSPLIT
        xb = X[b]
        if half:
            # roll so this core's queries are rows [0:NQ); key set is unchanged
            xb = np.concatenate([xb[NQ:], xb[:NQ]], axis=0)
        in_maps.append({"X": xb, "Wq": Wq, "Wk": Wk, "Wv": Wv})

    res = run_bass_kernel_spmd(
        nc, in_maps, core_ids=list(range(NCORES)), trace=trace
    )
    out = np.empty((B, N, H), dtype=np.float32)
    for core in range(NCORES):
        b, half = core // QSPLIT, core % QSPLIT
        out[b, half * NQ:(half + 1) * NQ] = res.results[core]["O"]
    if trace:
        return out, res
    return out


# revision 31
# speedup vs baseline: 2.1106x; 2.1106x over previous
"""Trainium2 Bass kernel for a single attention head.

Problem: X[4,4096,1024], Wq/Wk/Wv[1024,128] ->
  softmax((X@Wq)(X@Wk)^T / sqrt(1024)) @ (X@Wv)   -> [4,4096,128]

Sharding: 8 cores = 4 batches x 2 query-halves. Each core receives the full
X of its batch (rolled so its query half is rows [0:2048)), computes K/V for
all 4096 keys and flash-style attention for its 2048 queries.

On-core algorithm (all matmuls bf16 inputs, fp32 PSUM accumulation):
  1. X -> bf16 (cast DMA) -> X^T via XBAR transpose-DMA.
  2. K^T[h,n], V^T[h,n], Q^T[h,q] projections; V^T -> V[k,h] via transpose-DMA.
  3. Transposed flash attention per 1024-query chunk:
       S^T[k,q] = K_tile @ Q^T   (PSUM)
       P^T = exp(S^T/32)         (ACT, bf16 out)
       O^T[h,q] += V_tile^T @ P^T  ;  l[1,q] += ones^T @ P^T
     Epilogue: PE-transpose O^T and l, scale by 1/l, DMA out.
"""

import numpy as np

B, N, D, H = 4, 4096, 1024, 128
NCORES = 8
QSPLIT = 2  # cores per batch (query halves)
NQ = N // QSPLIT
SCALE = 1.0 / float(np.sqrt(np.float32(D)))
P = 128  # partitions
FB = 512  # matmul free-dim block (one fp32 PSUM bank)


def emit_attention(tc, X, Wq, Wk, Wv, O, n=N, d=D, nq=NQ, qc=1024):
    """Emit the single-core attention program into TileContext tc.

    X: [n, d] f32 DRAM (queries are rows [0:nq)); W*: [d, H] f32; O: [nq, H] f32.
    """
    import concourse.mybir as mybir
    from concourse.masks import make_identity

    nc = tc.nc
    dt = mybir.dt
    f32, bf16 = dt.float32, dt.bfloat16
    AF = mybir.ActivationFunctionType

    DT = d // P   # d tiles (contraction tiles for projections)
    NT = n // P   # key tiles
    qc = min(qc, nq)
    QB = qc // P  # 128-query blocks per chunk
    CR = min(FB, n)  # X rows per cast/transpose chunk (== FB for layout)
    NC = n // CR     # number of chunks
    assert nq % qc == 0 and d % P == 0 and n % CR == 0 and qc % P == 0

    from contextlib import ExitStack

    with ExitStack() as ctx:
        cpool = ctx.enter_context(tc.tile_pool(name="const", bufs=1))
        big = ctx.enter_context(tc.tile_pool(name="big", bufs=1))
        ptp = ctx.enter_context(tc.tile_pool(name="pt", bufs=3))
        epp = ctx.enter_context(tc.tile_pool(name="ep", bufs=2))
        accsb = ctx.enter_context(tc.tile_pool(name="accsb", bufs=2))

        ident = cpool.tile([P, P], f32)
        make_identity(nc, ident[:])
        ones_f = cpool.tile([P, 1], f32)
        nc.gpsimd.memset(ones_f[:], 1.0)

        w_sb = {}
        for name, w in (("wq", Wq), ("wk", Wk), ("wv", Wv)):
            t = cpool.tile([P, DT * H], bf16, tag=name)
            nc.gpsimd.dma_start(
                t[:].rearrange("p (t h) -> p t h", t=DT),
                w.rearrange("(t p) h -> p t h", p=P),
            )
            w_sb[name] = t

        xt = big.tile([P, DT * n], bf16)    # X^T: [d%128, dt*n + ncol]
        kT = big.tile([P, n], bf16)         # K^T[h, n]
        qT = big.tile([P, nq], bf16)        # Q^T[h, q]
        vT = big.tile([P, n], bf16)         # V^T[h, n] (staging)
        v_sb = big.tile([P, NT * H], bf16)  # V[k%128, kt*H + h]

        # ---- Phases 1+2: cast X to bf16 in DRAM, big chunked xbar
        # DMA-transposes into X^T (chunk-major layout: xt[p, c*DT*CR +
        # dt*CR + nb] = X^T[dt*128+p, c*CR+nb]), then projections.
        xbf_dram = nc.dram_tensor(
            "xbf_scratch", [n, d], bf16, kind="Internal"
        ).ap()
        vt_dram = nc.dram_tensor(
            "vt_scratch", [P, n], bf16, kind="Internal"
        ).ap()
        with tc.tile_pool(name="p12", bufs=3, space="PSUM") as p12:
            xt4 = xt[:].rearrange("p (c t nb) -> p c t nb", t=DT, nb=CR)
            for c in range(NC):
                nc.gpsimd.dma_start(
                    xbf_dram[c * CR:(c + 1) * CR, :],
                    X[c * CR:(c + 1) * CR, :],
                )
                nc.sync.dma_start_transpose(
                    xt4[:, c], xbf_dram[c * CR:(c + 1) * CR, :]
                )

            def project(wname, dst, ncols):
                c = 0
                while c * CR < ncols:
                    w = min(CR, ncols - c * CR)
                    ps = p12.tile([P, CR], f32, tag="pps")
                    for t in range(DT):
                        base = (c * DT + t) * CR
                        nc.tensor.matmul(
                            ps[:, :w],
                            w_sb[wname][:, t * H:(t + 1) * H],
                            xt[:, base:base + w],
                            start=(t == 0),
                            stop=(t == DT - 1),
                        )
                    nc.vector.tensor_copy(
                        dst[:, c * CR:c * CR + w], ps[:, :w]
                    )
                    c += 1

            project("wk", kT, n)
            project("wv", vT, n)
            project("wq", qT, nq)

            # V^T -> V: stage V^T to DRAM, one big transpose-load back
            nc.sync.dma_start(vt_dram[:, :], vT[:])
            nc.sync.dma_start_transpose(
                v_sb[:].rearrange("p (kt h) -> p kt h", h=H),
                vt_dram[:, :],
            )

        # ---- Phase 3: attention ----
        with ExitStack() as actx:
            stp = actx.enter_context(tc.tile_pool(name="stps", bufs=2, space="PSUM"))
            accp = actx.enter_context(tc.tile_pool(name="accps", bufs=1, space="PSUM"))

            for q0 in range(0, nq, qc):
                out_ps = accp.tile([P, qc], f32, tag="out")
                l_ps = accp.tile([1, qc], f32, tag="l")
                acc = None
                for kt in range(NT):
                    st = stp.tile([P, qc], f32, tag="st")
                    for j in range(0, qc, FB):
                        w = min(FB, qc - j)
                        nc.tensor.matmul(
                            st[:, j:j + w],
                            kT[:, kt * P:(kt + 1) * P],
                            qT[:, q0 + j: q0 + j + w],
                            start=True, stop=True,
                        )
                    pT = ptp.tile([P, qc], bf16, tag="pt")
                    nc.scalar.activation(pT[:], st[:], AF.Exp, scale=SCALE)
                    for j in range(0, qc, FB):
                        w = min(FB, qc - j)
                        nc.tensor.matmul(
                            out_ps[:, j:j + w],
                            v_sb[:, kt * H:(kt + 1) * H],
                            pT[:, j:j + w],
                            start=(kt == 0), stop=(kt == NT - 1),
                        )
                    # softmax denominator: accumulate P^T on DVE (f32),
                    # reduced over partitions by one small matmul at the end
                    nacc = accsb.tile([P, qc], f32, tag="acc")
                    if kt == 0:
                        nc.vector.tensor_copy(nacc[:], pT[:])
                    else:
                        nc.vector.tensor_add(nacc[:], acc[:], pT[:])
                    acc = nacc
                for j in range(0, qc, FB):
                    w = min(FB, qc - j)
                    nc.tensor.matmul(
                        l_ps[:, j:j + w], ones_f[:], acc[:, j:j + w],
                        start=True, stop=True,
                    )

                # epilogue: 1/l, transpose O^T -> O, scale, store
                l_sb = epp.tile([1, qc], f32, tag="lsb")
                nc.vector.tensor_copy(l_sb[:], l_ps[:])
                r_sb = epp.tile([P, QB], f32, tag="rsb")
                for blk in range(QB):
                    lt = stp.tile([P, 1], f32, tag="st")
                    nc.tensor.transpose(
                        lt[:], l_sb[:, blk * P:(blk + 1) * P], ident[:1, :1]
                    )
                    nc.vector.reciprocal(r_sb[:, blk:blk + 1], lt[:])
                ob = epp.tile([P, qc], f32, tag="ob")
                nc.vector.tensor_copy(ob[:], out_ps[:])
                o_sb = epp.tile([P, QB * H], f32, tag="osb")
                for blk in range(QB):
                    ot = stp.tile([P, P], f32, tag="st")
                    nc.tensor.transpose(ot[:], ob[:, blk * P:(blk + 1) * P], ident[:])
                    nc.scalar.mul(
                        o_sb[:, blk * H:(blk + 1) * H], ot[:], r_sb[:, blk:blk + 1]
                    )
                nc.sync.dma_start(
                    O[q0:q0 + qc, :].rearrange("(qb p) h -> p qb h", p=P),
                    o_sb[:].rearrange("p (qb h) -> p qb h", qb=QB),
                )


def build_bass(n=N, d=D, nq=NQ, qc=1024):
    import concourse.mybir as mybir
    from concourse import bacc
    from concourse.tile import TileContext

    dt = mybir.dt
    nc = bacc.Bacc("TRN2", target_bir_lowering=False, debug=False)
    X = nc.dram_tensor("X", [n, d], dt.float32, kind="ExternalInput").ap()
    Wq = nc.dram_tensor("Wq", [d, H], dt.float32, kind="ExternalInput").ap()
    Wk = nc.dram_tensor("Wk", [d, H], dt.float32, kind="ExternalInput").ap()
    Wv = nc.dram_tensor("Wv", [d, H], dt.float32, kind="ExternalInput").ap()
    O = nc.dram_tensor("O", [nq, H], dt.float32, kind="ExternalOutput").ap()

    with TileContext(nc) as tc:
        emit_attention(tc, X, Wq, Wk, Wv, O, n=n, d=d, nq=nq, qc=qc)
    nc.compile()  # bacc passes: split multi-waits into EVSEM chains, etc.
    return nc


_CACHED = {}


def _get_nc():
    if "nc" not in _CACHED:
        _CACHED["nc"] = build_bass()
    return _CACHED["nc"]


def kernel(X, Wq, Wk, Wv, trace=False):
    """Full-input entry point: X [4,4096,1024] f32 -> [4,4096,128] f32."""
    from concourse.bass_utils import run_bass_kernel_spmd

    X = np.ascontiguousarray(X, dtype=np.float32)
    Wq = np.ascontiguousarray(Wq, dtype=np.float32)
    Wk = np.ascontiguousarray(Wk, dtype=np.float32)
    Wv = np.ascontiguousarray(Wv, dtype=np.float32)

    nc = _get_nc()
    in_maps = []
    for core in range(NCORES):
        b, half = core // QSPLIT, core % QSPLIT
        xb = X[b]
        if half:
            # roll so this core's queries are rows [0:NQ); key set is unchanged
            xb = np.concatenate([xb[NQ:], xb[:NQ]], axis=0)
        in_maps.append({"X": xb, "Wq": Wq, "Wk": Wk, "Wv": Wv})

    res = run_bass_kernel_spmd(
        nc, in_maps, core_ids=list(range(NCORES)), trace=trace
    )
    out = np.empty((B, N, H), dtype=np.float32)
    for core in range(NCORES):
        b, half = core // QSPLIT, core % QSPLIT
        out[b, half * NQ:(half + 1) * NQ] = res.results[core]["O"]
    if trace:
        return out, res
    return out


# revision 36
# speedup vs baseline: 2.1616x; 1.0242x over previous
"""Trainium2 Bass kernel for a single attention head.

Problem: X[4,4096,1024], Wq/Wk/Wv[1024,128] ->
  softmax((X@Wq)(X@Wk)^T / sqrt(1024)) @ (X@Wv)   -> [4,4096,128]

Sharding: 8 cores = 4 batches x 2 query-halves. Each core receives the full
X of its batch (rolled so its query half is rows [0:2048)), computes K/V for
all 4096 keys and flash-style attention for its 2048 queries.

On-core algorithm (all matmuls bf16 inputs, fp32 PSUM accumulation):
  1. X -> bf16 (cast DMA) -> X^T via XBAR transpose-DMA.
  2. K^T[h,n], V^T[h,n], Q^T[h,q] projections; V^T -> V[k,h] via transpose-DMA.
  3. Transposed flash attention per 1024-query chunk:
       S^T[k,q] = K_tile @ Q^T   (PSUM)
       P^T = exp(S^T/32)         (ACT, bf16 out)
       O^T[h,q] += V_tile^T @ P^T  ;  l[1,q] += ones^T @ P^T
     Epilogue: PE-transpose O^T and l, scale by 1/l, DMA out.
"""

import numpy as np

B, N, D, H = 4, 4096, 1024, 128
NCORES = 8
QSPLIT = 2  # cores per batch (query halves)
NQ = N // QSPLIT
SCALE = 1.0 / float(np.sqrt(np.float32(D)))
P = 128  # partitions
FB = 512  # matmul free-dim block (one fp32 PSUM bank)


def emit_attention(tc, X, Wq, Wk, Wv, O, n=N, d=D, nq=NQ, qc=1024):
    """Emit the single-core attention program into TileContext tc.

    X: [n, d] f32 DRAM (queries are rows [0:nq)); W*: [d, H] f32; O: [nq, H] f32.
    """
    import concourse.mybir as mybir
    from concourse.masks import make_identity

    nc = tc.nc
    dt = mybir.dt
    f32, bf16 = dt.float32, dt.bfloat16
    AF = mybir.ActivationFunctionType

    DT = d // P   # d tiles (contraction tiles for projections)
    NT = n // P   # key tiles
    qc = min(qc, nq)
    QB = qc // P  # 128-query blocks per chunk
    CR = min(FB, n)  # X rows per cast/transpose chunk (== FB for layout)
    NC = n // CR     # number of chunks
    assert nq % qc == 0 and d % P == 0 and n % CR == 0 and qc % P == 0

    from contextlib import ExitStack

    with ExitStack() as ctx:
        cpool = ctx.enter_context(tc.tile_pool(name="const", bufs=1))
        big = ctx.enter_context(tc.tile_pool(name="big", bufs=1))
        ptp = ctx.enter_context(tc.tile_pool(name="pt", bufs=3))
        epp = ctx.enter_context(tc.tile_pool(name="ep", bufs=2))
        accsb = ctx.enter_context(tc.tile_pool(name="accsb", bufs=2))
        # all PSUM pools coexist (8 banks total) so projections and the
        # attention k-loop can overlap without pool-boundary serialization
        p12 = ctx.enter_context(tc.tile_pool(name="p12", bufs=2, space="PSUM"))
        stp = ctx.enter_context(tc.tile_pool(name="stps", bufs=2, space="PSUM"))
        accp = ctx.enter_context(tc.tile_pool(name="accps", bufs=1, space="PSUM"))

        ident = cpool.tile([P, P], f32)
        make_identity(nc, ident[:])
        ones_f = cpool.tile([P, 1], f32)
        nc.gpsimd.memset(ones_f[:], 1.0)

        w_sb = {}
        for name, w in (("wq", Wq), ("wk", Wk), ("wv", Wv)):
            t = cpool.tile([P, DT * H], bf16, tag=name)
            nc.gpsimd.dma_start(
                t[:].rearrange("p (t h) -> p t h", t=DT),
                w.rearrange("(t p) h -> p t h", p=P),
            )
            w_sb[name] = t

        xt = big.tile([P, DT * n], bf16)    # X^T: [d%128, dt*n + ncol]
        kT = big.tile([P, n], bf16)         # K^T[h, n]
        qT = big.tile([P, nq], bf16)        # Q^T[h, q]
        vT = big.tile([P, n], bf16)         # V^T[h, n] (staging)
        v_sb = big.tile([P, NT * H], bf16)  # V[k%128, kt*H + h]

        # ---- Phases 1+2: cast X to bf16 in DRAM, big chunked xbar
        # DMA-transposes into X^T (chunk-major layout: xt[p, c*DT*CR +
        # dt*CR + nb] = X^T[dt*128+p, c*CR+nb]), then projections.
        xbf_dram = nc.dram_tensor(
            "xbf_scratch", [n, d], bf16, kind="Internal"
        ).ap()
        xt4 = xt[:].rearrange("p (c t nb) -> p c t nb", t=DT, nb=CR)
        for c in range(NC):
            nc.gpsimd.dma_start(
                xbf_dram[c * CR:(c + 1) * CR, :],
                X[c * CR:(c + 1) * CR, :],
            )
            nc.sync.dma_start_transpose(
                xt4[:, c], xbf_dram[c * CR:(c + 1) * CR, :]
            )

        def project(wname, dst, ncols, c):
            w = min(CR, ncols - c * CR)
            ps = p12.tile([P, CR], f32, tag="pps")
            for t in range(DT):
                base = (c * DT + t) * CR
                nc.tensor.matmul(
                    ps[:, :w],
                    w_sb[wname][:, t * H:(t + 1) * H],
                    xt[:, base:base + w],
                    start=(t == 0),
                    stop=(t == DT - 1),
                )
            nc.vector.tensor_copy(dst[:, c * CR:c * CR + w], ps[:, :w])

        v_sb3 = v_sb[:].rearrange("p (kt h) -> p kt h", h=H)
        KPC = CR // P  # key tiles per chunk
        for c in range(NC):
            project("wk", kT, n, c)
            project("wv", vT, n, c)
            if c * CR < nq:
                project("wq", qT, nq, c)
            # V^T chunk -> V[k, h] (SBUF->SBUF xbar transpose)
            nc.sync.dma_start_transpose(
                v_sb3[:, c * KPC:(c + 1) * KPC],
                vT[:, c * CR:(c + 1) * CR],
            )

        # ---- Phase 3: attention ----
        if True:
            for q0 in range(0, nq, qc):
                out_ps = accp.tile([P, qc], f32, tag="out")
                l_ps = stp.tile([1, qc], f32, tag="st")
                acc = None
                for kt in range(NT):
                    st = stp.tile([P, qc], f32, tag="st")
                    for j in range(0, qc, FB):
                        w = min(FB, qc - j)
                        nc.tensor.matmul(
                            st[:, j:j + w],
                            kT[:, kt * P:(kt + 1) * P],
                            qT[:, q0 + j: q0 + j + w],
                            start=True, stop=True,
                        )
                    pT = ptp.tile([P, qc], bf16, tag="pt")
                    nc.scalar.activation(pT[:], st[:], AF.Exp, scale=SCALE)
                    for j in range(0, qc, FB):
                        w = min(FB, qc - j)
                        nc.tensor.matmul(
                            out_ps[:, j:j + w],
                            v_sb[:, kt * H:(kt + 1) * H],
                            pT[:, j:j + w],
                            start=(kt == 0), stop=(kt == NT - 1),
                        )
                    # softmax denominator: accumulate P^T on DVE (f32),
                    # reduced over partitions by one small matmul at the end
                    nacc = accsb.tile([P, qc], f32, tag="acc")
                    if kt == 0:
                        nc.vector.tensor_copy(nacc[:], pT[:])
                    else:
                        nc.vector.tensor_add(nacc[:], acc[:], pT[:])
                    acc = nacc
                for j in range(0, qc, FB):
                    w = min(FB, qc - j)
                    nc.tensor.matmul(
                        l_ps[:, j:j + w], ones_f[:], acc[:, j:j + w],
                        start=True, stop=True,
                    )

                # epilogue: 1/l, transpose O^T -> O, scale, store
                l_sb = epp.tile([1, qc], f32, tag="lsb")
                nc.vector.tensor_copy(l_sb[:], l_ps[:])
                r_sb = epp.tile([P, QB], f32, tag="rsb")
                for blk in range(QB):
                    lt = stp.tile([P, 1], f32, tag="st")
                    nc.tensor.transpose(
                        lt[:], l_sb[:, blk * P:(blk + 1) * P], ident[:1, :1]
                    )
                    nc.vector.reciprocal(r_sb[:, blk:blk + 1], lt[:])
                ob = epp.tile([P, qc], f32, tag="ob")
                nc.vector.tensor_copy(ob[:], out_ps[:])
                o_sb = epp.tile([P, QB * H], f32, tag="osb")
                for blk in range(QB):
                    ot = stp.tile([P, P], f32, tag="st")
                    nc.tensor.transpose(ot[:], ob[:, blk * P:(blk + 1) * P], ident[:])
                    nc.scalar.mul(
                        o_sb[:, blk * H:(blk + 1) * H], ot[:], r_sb[:, blk:blk + 1]
                    )
                nc.sync.dma_start(
                    O[q0:q0 + qc, :].rearrange("(qb p) h -> p qb h", p=P),
                    o_sb[:].rearrange("p (qb h) -> p qb h", qb=QB),
                )


def build_bass(n=N, d=D, nq=NQ, qc=1024):
    import concourse.mybir as mybir
    from concourse import bacc
    from concourse.tile import TileContext

    dt = mybir.dt
    nc = bacc.Bacc("TRN2", target_bir_lowering=False, debug=False)
    X = nc.dram_tensor("X", [n, d], dt.float32, kind="ExternalInput").ap()
    Wq = nc.dram_tensor("Wq", [d, H], dt.float32, kind="ExternalInput").ap()
    Wk = nc.dram_tensor("Wk", [d, H], dt.float32, kind="ExternalInput").ap()
    Wv = nc.dram_tensor("Wv", [d, H], dt.float32, kind="ExternalInput").ap()
    O = nc.dram_tensor("O", [nq, H], dt.float32, kind="ExternalOutput").ap()

    with TileContext(nc) as tc:
        emit_attention(tc, X, Wq, Wk, Wv, O, n=n, d=d, nq=nq, qc=qc)
    nc.compile()  # bacc passes: split multi-waits into EVSEM chains, etc.
    return nc


_CACHED = {}


def _get_nc():
    if "nc" not in _CACHED:
        _CACHED["nc"] = build_bass()
    return _CACHED["nc"]


def kernel(X, Wq, Wk, Wv, trace=False):
    """Full-input entry point: X [4,4096,1024] f32 -> [4,4096,128] f32."""
    from concourse.bass_utils import run_bass_kernel_spmd

    X = np.ascontiguousarray(X, dtype=np.float32)
    Wq = np.ascontiguousarray(Wq, dtype=np.float32)
    Wk = np.ascontiguousarray(Wk, dtype=np.float32)
    Wv = np.ascontiguousarray(Wv, dtype=np.float32)

    nc = _get_nc()
    in_maps = []
    for core in range(NCORES):
        b, half = core // QSPLIT, core % QSPLIT
        xb = X[b]
        if half:
            # roll so this core's queries are rows [0:NQ); key set is unchanged
            xb = np.concatenate([xb[NQ:], xb[:NQ]], axis=0)
        in_maps.append({"X": xb, "Wq": Wq, "Wk": Wk, "Wv": Wv})

    res = run_bass_kernel_spmd(
        nc, in_maps, core_ids=list(range(NCORES)), trace=trace
    )
    out = np.empty((B, N, H), dtype=np.float32)
    for core in range(NCORES):
        b, half = core // QSPLIT, core % QSPLIT
        out[b, half * NQ:(half + 1) * NQ] = res.results[core]["O"]
    if trace:
        return out, res
    return out
